# revision 7
# baseline (speedup 1.0000x reference)
"""Doc-causal self-attention (B=1, T=4096, DIM=1024, H=8, HD=128) on 8 TRN2
NeuronCores.

Strategy: sequence-parallel with doc-aligned chunks. docs is sorted, so T
splits into ~8 documents; each core takes one contiguous chunk of rows cut at
(or near) document boundaries. Doc-causal attention never crosses a document,
so each core's rows attend only to rows inside its own chunk context (chunk +
intra-doc prefix when a cut lands mid-document). Every core independently
computes qkv projection, rms-norm, rotary, masked softmax attention and its
c_proj output rows -- zero inter-core collectives. Host gathers per-core
output chunks.

Numerics: matmuls run in float32r (TF32, full TensorE rate at N>=256);
host pre-rounds matmul inputs to TF32. Softmax skips max-subtraction
(|logits| <= 0.12*128 by Cauchy-Schwarz after rms-norm). Masking is
multiplicative on exp(scores): mask value 0/1, applied post-exp.
rms scale for q is folded into q; for k it is applied per-partition via the
activation scale operand of the exp (ACT computes exp(in*scale)).
"""
import contextlib
import ctypes
import sys
import types

import numpy as np

T, DIM, H, HD = 4096, 1024, 8, 128
NCORE = 8
ATTN_SCALE = 0.12
EPS = float(np.finfo(np.float32).eps)

_CACHE = {}


def _round_tf32(a):
    u = np.ascontiguousarray(a, dtype=np.float32).view(np.uint32).astype(np.uint64)
    r = ((u + 0x1000 + ((u >> 13) & 1)) & ~np.uint64(0x1FFF)).astype(np.uint32)
    return r.view(np.float32)


# ---------------------------------------------------------------- ntff shim
def _install_ntff_shim():
    try:
        import antenv.axon_hooks  # noqa: F401
        return
    except ImportError:
        pass
    import antenv

    so_path = "/opt/axon/libaxon_pjrt.so"

    def _hook_factory():
        lib = ctypes.CDLL(so_path)
        if not hasattr(lib, "axon_start_nrt_profile"):
            return None
        lib.axon_start_nrt_profile.argtypes = [ctypes.POINTER(ctypes.c_int64), ctypes.c_size_t]
        lib.axon_start_nrt_profile.restype = ctypes.c_int64
        lib.axon_stop_nrt_profile.argtypes = [ctypes.c_char_p]
        lib.axon_stop_nrt_profile.restype = ctypes.c_int64

        @contextlib.contextmanager
        def _hook(output_dir, device_ids):
            import jax

            jax.devices()
            if device_ids:
                ids = (ctypes.c_int64 * len(device_ids))(*device_ids)
                rc = lib.axon_start_nrt_profile(ids, len(device_ids))
            else:
                rc = lib.axon_start_nrt_profile(None, 0)
            if rc != 0:
                raise RuntimeError(f"axon_start_nrt_profile rc={rc}")
            try:
                yield
            finally:
                n = lib.axon_stop_nrt_profile(str(output_dir).encode())
                print(f"profile: {n} file(s) written to {output_dir}", file=sys.stderr)

        return _hook

    mod = types.ModuleType("antenv.axon_hooks")
    _h = [_hook_factory()]
    mod.set_axon_ntff_profile_hook = lambda h: _h.__setitem__(0, h)
    mod.get_axon_ntff_profile_hook = lambda: _h[0]
    sys.modules["antenv.axon_hooks"] = mod
    antenv.axon_hooks = mod


# ---------------------------------------------------------------- planning
def _ceil128(n):
    return ((n + 127) // 128) * 128


def _plan(docs):
    docs = np.asarray(docs)
    bounds = [0] + list((np.where(np.diff(docs) != 0)[0] + 1).tolist()) + [T]
    doc_start = np.zeros(T, np.int64)
    for b0, b1 in zip(bounds[:-1], bounds[1:]):
        doc_start[b0:b1] = b0
    base = T // NCORE
    cuts = [0]
    for c in range(1, NCORE):
        tgt = c * base
        b = min(bounds, key=lambda v: abs(v - tgt))
        s = b if abs(b - tgt) <= 192 else tgt
        s = int(min(max(s, cuts[-1] + 1), T - 1))
        cuts.append(s)
    cuts.append(T)
    chunks = [(cuts[i], cuts[i + 1]) for i in range(NCORE)]
    ctx_starts = [int(doc_start[s]) for s, _ in chunks]
    MAXQ = _ceil128(max(e - s for s, e in chunks))
    PFX = _ceil128(max(s - cs for (s, _), cs in zip(chunks, ctx_starts)))
    MAXC = MAXQ + PFX
    CO = PFX
    return {"chunks": chunks, "ctx_starts": ctx_starts, "MAXQ": MAXQ,
            "MAXC": MAXC, "CO": CO, "doc_start": doc_start}


def _nchunks(width):
    n = (width + 511) // 512
    while width % n:
        n += 1
    return n, width // n


# ---------------------------------------------------------------- device
def _build(MAXQ, MAXC, CO):
    import concourse.bacc as bacc
    import concourse.mybir as mybir
    from concourse import tile, bass_isa

    dt = mybir.dt
    AF = mybir.ActivationFunctionType
    QT = MAXQ // 128
    TC = MAXC // 128
    NCH, CH = _nchunks(MAXQ)     # q-side chunks (e.g. 2 x 320)
    NCC, CC = _nchunks(MAXC)     # ctx-side chunks
    PSW = 512 * max(NCH, NCC, 2)  # psum tile free width (bank-strided chunks)

    nc = bacc.Bacc("TRN2", target_bir_lowering=False)

    xT_d = nc.dram_tensor("xT", [DIM, MAXC], dt.float32r, kind="ExternalInput")
    wq_d = nc.dram_tensor("wq", [DIM, DIM], dt.float32r, kind="ExternalInput")
    wk_d = nc.dram_tensor("wk", [DIM, DIM], dt.float32r, kind="ExternalInput")
    wv_d = nc.dram_tensor("wv", [DIM, DIM], dt.float32r, kind="ExternalInput")
    cw_d = nc.dram_tensor("cw", [DIM, DIM], dt.bfloat16, kind="ExternalInput")
    ve_d = nc.dram_tensor("ve", [MAXC, DIM], dt.float32, kind="ExternalInput")
    cos_d = nc.dram_tensor("cosT", [128, MAXC], dt.float32, kind="ExternalInput")
    sin_d = nc.dram_tensor("sinT", [128, MAXC], dt.float32, kind="ExternalInput")
    msk_d = nc.dram_tensor("mask", [MAXC, MAXQ], dt.bfloat16, kind="ExternalInput")
    c12_d = nc.dram_tensor("c012", [1, 1], dt.float32, kind="ExternalInput")
    eps_d = nc.dram_tensor("epsb", [128, 1], dt.float32, kind="ExternalInput")
    out_d = nc.dram_tensor("outT", [DIM, MAXQ], dt.float32, kind="ExternalOutput")

    add, mult, sub = mybir.AluOpType.add, mybir.AluOpType.mult, mybir.AluOpType.subtract

    with tile.TileContext(nc) as tc:
        with contextlib.ExitStack() as ex:
            pers = ex.enter_context(tc.tile_pool(name="pers", bufs=1))
            strm = ex.enter_context(tc.tile_pool(name="strm", bufs=4))
            nrm = ex.enter_context(tc.tile_pool(name="nrm", bufs=2))
            petp = ex.enter_context(tc.tile_pool(name="et", bufs=3))
            pps = ex.enter_context(tc.tile_pool(name="ps", bufs=3, space="PSUM"))
            pyy = ex.enter_context(tc.tile_pool(name="py", bufs=1, space="PSUM"))

            # ---- persistent loads
            xs = []
            for c in range(8):
                t = pers.tile([128, MAXC], dt.float32r, tag=f"x{c}")
                nc.sync.dma_start(t[:], xT_d[c * 128:(c + 1) * 128, :])
                xs.append(t)
            cosT = pers.tile([128, MAXC], dt.float32, tag="cos")
            sinT = pers.tile([128, MAXC], dt.float32, tag="sin")
            nc.sync.dma_start(cosT[:], cos_d[:])
            nc.sync.dma_start(sinT[:], sin_d[:])
            c12 = pers.tile([1, 1], dt.float32, tag="c12")
            nc.sync.dma_start(c12[:], c12_d[:])
            epsb = pers.tile([128, 1], dt.float32, tag="epsb")
            nc.sync.dma_start(epsb[:], eps_d[:])
            msks = []
            for jt in range(TC):
                t = pers.tile([128, MAXQ], dt.bfloat16, tag=f"m{jt}")
                nc.sync.dma_start(t[:], msk_d[jt * 128:(jt + 1) * 128, :])
                msks.append(t)

            qrs = [pers.tile([128, MAXQ], dt.float32r, tag=f"qr{h}", name=f"qr{h}") for h in range(H)]
            krs = [pers.tile([128, MAXC], dt.float32r, tag=f"kr{h}", name=f"kr{h}") for h in range(H)]
            vsb = [pers.tile([128, DIM], dt.bfloat16, tag=f"v{jt}", name=f"v{jt}") for jt in range(TC)]
            yts = [pers.tile([128, MAXQ], dt.bfloat16, tag=f"y{h}", name=f"yt{h}") for h in range(H)]
            sks = [pers.tile([128, TC], dt.float32, tag=f"sk{h}", name=f"sk{h}") for h in range(H)]
            skreps = [pers.tile([1, MAXC], dt.float32, tag=f"skr{h}", name=f"skr{h}") for h in range(H)]

            # ---- qk projection + norm + rotary, per head
            for h in range(H):
                with nc.named_scope(f"proj{h}"):
                    qps = pps.tile([128, PSW], dt.float32, tag="mm")
                    kps = pps.tile([128, PSW], dt.float32, tag="mm")
                    for c in range(8):
                        wqt = strm.tile([128, 128], dt.float32r, tag="w")
                        nc.sync.dma_start(wqt[:], wq_d[c * 128:(c + 1) * 128, h * 128:(h + 1) * 128])
                        for b in range(NCH):
                            nc.tensor.matmul(qps[:, b * 512:b * 512 + CH], wqt[:],
                                             xs[c][:, CO + b * CH:CO + (b + 1) * CH],
                                             start=(c == 0), stop=(c == 7))
                    for c in range(8):
                        wkt = strm.tile([128, 128], dt.float32r, tag="w")
                        nc.sync.dma_start(wkt[:], wk_d[c * 128:(c + 1) * 128, h * 128:(h + 1) * 128])
                        for b in range(NCC):
                            nc.tensor.matmul(kps[:, b * 512:b * 512 + CC], wkt[:],
                                             xs[c][:, b * CC:(b + 1) * CC],
                                             start=(c == 0), stop=(c == 7))
                with nc.named_scope(f"norm{h}"):
                    # q: rms scale folded into q before rotary
                    sq = nrm.tile([128, MAXQ], dt.float32, tag="sq")
                    for b in range(NCH):
                        nc.scalar.activation(sq[:, b * CH:(b + 1) * CH],
                                             qps[:, b * 512:b * 512 + CH], AF.Square)
                    ssq = nrm.tile([128, MAXQ], dt.float32, tag="ssq")
                    nc.gpsimd.partition_all_reduce(ssq[:], sq[:], channels=128,
                                                   reduce_op=bass_isa.ReduceOp.add)
                    rms = nrm.tile([128, MAXQ], dt.float32, tag="rts")
                    nc.scalar.activation(rms[:], ssq[:], AF.Sqrt, bias=epsb[:])
                    nc.vector.reciprocal_approx_fast(out=rms[:], in_=rms[:])
                    qs = nrm.tile([128, MAXQ], dt.float32, tag="sq")
                    for b in range(NCH):
                        nc.vector.tensor_mul(qs[:, b * CH:(b + 1) * CH],
                                             qps[:, b * 512:b * 512 + CH],
                                             rms[:, b * CH:(b + 1) * CH])
                    qsw = nrm.tile([128, MAXQ], dt.float32, tag="qsw")
                    nc.sync.dma_start(qsw[0:64, :], qs[64:128, :])
                    nc.sync.dma_start(qsw[64:128, :], qs[0:64, :])
                    ta = nrm.tile([128, MAXC], dt.float32, tag="ta")
                    tb = nrm.tile([128, MAXC], dt.float32, tag="tb")
                    nc.vector.tensor_mul(ta[:, 0:MAXQ], qs[:], cosT[:, CO:CO + MAXQ])
                    nc.vector.tensor_mul(tb[:, 0:MAXQ], qsw[:], sinT[:, CO:CO + MAXQ])
                    nc.vector.tensor_add(qrs[h][:], ta[:, 0:MAXQ], tb[:, 0:MAXQ])
                    # k: evacuate, square, rms kept separate (applied at exp)
                    ks = nrm.tile([128, MAXC], dt.float32, tag="ks", bufs=1)
                    for b in range(NCC):
                        nc.scalar.activation(ks[:, b * CC:(b + 1) * CC],
                                             kps[:, b * 512:b * 512 + CC], AF.Copy)
                    sqk = nrm.tile([128, MAXC], dt.float32, tag="sqk", bufs=1)
                    nc.scalar.activation(sqk[:], ks[:], AF.Square)
                    ssqk = nrm.tile([128, MAXC], dt.float32, tag="ssqk", bufs=1)
                    nc.gpsimd.partition_all_reduce(ssqk[:], sqk[:], channels=128,
                                                   reduce_op=bass_isa.ReduceOp.add)
                    skf = nrm.tile([128, MAXC], dt.float32, tag="rtsk", bufs=1)
                    nc.scalar.activation(skf[:], ssqk[:], AF.Sqrt, bias=epsb[:])
                    nc.vector.reciprocal_approx_fast(out=skf[:], in_=skf[:])
                    nc.scalar.activation(skreps[h][:], skf[0:1, :], AF.Copy)
                    ksw = nrm.tile([128, MAXC], dt.float32, tag="ksw")
                    nc.sync.dma_start(ksw[0:64, :], ks[64:128, :])
                    nc.sync.dma_start(ksw[64:128, :], ks[0:64, :])
                    ka = nrm.tile([128, MAXC], dt.float32, tag="ta")
                    kb = nrm.tile([128, MAXC], dt.float32, tag="tb")
                    nc.vector.tensor_mul(ka[:], ks[:], cosT[:])
                    nc.vector.tensor_mul(kb[:], ksw[:], sinT[:])
                    nc.vector.tensor_add(krs[h][:], ka[:], kb[:])

            # ---- v projection + ve lerp (2 jt-groups, streamed wv)
            with nc.named_scope("vproj"):
                groups = [list(range(TC))[i::2] for i in range(2)]
                groups = [list(range(TC))[:(TC + 1) // 2], list(range(TC))[(TC + 1) // 2:]]
                for grp in groups:
                    vpss = [pps.tile([128, PSW], dt.float32, tag="mm", name=f"vps{jt}") for jt in grp]
                    for c in range(8):
                        wvt = strm.tile([128, DIM], dt.float32r, tag="wv", bufs=2)
                        nc.sync.dma_start(wvt[:], wv_d[c * 128:(c + 1) * 128, :])
                        for vps, jt in zip(vpss, grp):
                            for b2 in range(2):
                                nc.tensor.matmul(vps[:, b2 * 512:(b2 + 1) * 512],
                                                 xs[c][:, jt * 128:(jt + 1) * 128],
                                                 wvt[:, b2 * 512:(b2 + 1) * 512],
                                                 start=(c == 0), stop=(c == 7))
                    for vps, jt in zip(vpss, grp):
                        vet = strm.tile([128, DIM], dt.float32, tag="ve", bufs=2)
                        nc.sync.dma_start(vet[:], ve_d[jt * 128:(jt + 1) * 128, :])
                        nc.vector.tensor_add(vsb[jt][:], vps[:, 0:1024], vet[:])

            # ---- s_k column transpose (PE), per head
            with nc.named_scope("sk"):
                for h in range(H):
                    skp = pps.tile([128, PSW], dt.float32, tag="mm")
                    for jt in range(TC):
                        nc.tensor.matmul(skp[:, jt:jt + 1], skreps[h][0:1, jt * 128:(jt + 1) * 128],
                                         c12[:], start=True, stop=True)
                    nc.vector.tensor_copy(out=sks[h][:], in_=skp[:, 0:TC])

            # ---- attention, per head (QK / mask-mul / exp / PV pipelined)
            for h in range(H):
                with nc.named_scope(f"attn{h}"):
                    yps = pyy.tile([128, PSW], dt.float32, tag="y")
                    eacc = petp.tile([128, MAXQ], dt.float32, tag="eacc")
                    ets = []
                    for jt in range(TC):
                        stp = pps.tile([128, PSW], dt.float32, tag="mm")
                        for b in range(NCH):
                            nc.tensor.matmul(stp[:, b * 512:b * 512 + CH],
                                             krs[h][:, jt * 128:(jt + 1) * 128],
                                             qrs[h][:, b * CH:(b + 1) * CH],
                                             start=True, stop=True)
                        et = petp.tile([128, MAXQ], dt.bfloat16, tag="et")
                        for b in range(NCH):
                            nc.scalar.activation(et[:, b * CH:(b + 1) * CH],
                                                 stp[:, b * 512:b * 512 + CH], AF.Exp,
                                                 scale=sks[h][:, jt:jt + 1])
                        nc.vector.tensor_mul(et[:], et[:], msks[jt][:])
                        if jt == 0:
                            nc.vector.tensor_copy(out=eacc[:], in_=et[:])
                        else:
                            nc.vector.tensor_add(eacc[:], eacc[:], et[:])
                        ets.append(et)
                        for b in range(NCH):
                            nc.tensor.matmul(yps[:, b * 512:b * 512 + CH],
                                             vsb[jt][:, h * 128:(h + 1) * 128],
                                             et[:, b * CH:(b + 1) * CH],
                                             start=(jt == 0), stop=(jt == TC - 1))
                    rsum = nrm.tile([128, MAXQ], dt.float32, tag="rsum")
                    nc.gpsimd.partition_all_reduce(rsum[:], eacc[:], channels=128,
                                                   reduce_op=bass_isa.ReduceOp.add)
                    rrec = rsum
                    nc.vector.reciprocal_approx_fast(out=rrec[:], in_=rrec[:])
                    for b in range(NCH):
                        nc.vector.tensor_mul(yts[h][:, b * CH:(b + 1) * CH],
                                             yps[:, b * 512:b * 512 + CH],
                                             rrec[:, b * CH:(b + 1) * CH])

            # ---- c_proj
            with nc.named_scope("cproj"):
                for e in range(8):
                    ops = pps.tile([128, PSW], dt.float32, tag="mm")
                    for h in range(H):
                        cwt = strm.tile([128, 128], dt.bfloat16, tag="cw")
                        nc.sync.dma_start(cwt[:], cw_d[h * 128:(h + 1) * 128, e * 128:(e + 1) * 128])
                        for b in range(NCH):
                            nc.tensor.matmul(ops[:, b * 512:b * 512 + CH], cwt[:],
                                             yts[h][:, b * CH:(b + 1) * CH],
                                             start=(h == 0), stop=(h == 7))
                    osb = nrm.tile([128, MAXQ], dt.float32, tag="osb")
                    for b in range(NCH):
                        nc.scalar.activation(osb[:, b * CH:(b + 1) * CH],
                                             ops[:, b * 512:b * 512 + CH], AF.Copy)
                    nc.sync.dma_start(out_d[e * 128:(e + 1) * 128, :], osb[:])

    nc.compile()
    return nc


# ---------------------------------------------------------------- host glue
def _prep(x, ve, qkv_w, lambdas, c_proj_w, plan):
    x = np.asarray(x, np.float32).reshape(T, DIM)
    ve = np.asarray(ve, np.float32).reshape(T, DIM)
    qkv_w = np.asarray(qkv_w, np.float32)
    lam0, lam1 = [float(v) for v in np.asarray(lambdas, np.float32)]
    MAXQ, MAXC, CO = plan["MAXQ"], plan["MAXC"], plan["CO"]
    docs = plan["docs"]

    import ml_dtypes
    bf16 = ml_dtypes.bfloat16
    wq = _round_tf32(qkv_w[0].T)
    wk = _round_tf32(qkv_w[1].T)
    wv = _round_tf32(qkv_w[2].T * lam0)
    cw = np.ascontiguousarray(np.asarray(c_proj_w, np.float32).T).astype(bf16)
    c012 = np.full((1, 1), ATTN_SCALE * HD, np.float32)

    # rotary tables (global positions; freqs as in reference)
    lin = np.linspace(0.0, 1.0, HD // 4).astype(np.float32)
    freqs = np.power(np.float32(1.0 / 1024.0), lin).astype(np.float32)
    freqs = np.concatenate([freqs, np.zeros(HD // 4, np.float32)])

    in_maps = []
    for (s, e), cs in zip(plan["chunks"], plan["ctx_starts"]):
        clen, plen = e - s, s - cs
        pos = np.full(MAXC, -1, np.int64)
        pos[CO - plen:CO + clen] = np.arange(cs, e)
        valid = pos >= 0
        rows = np.where(valid, pos, 0)

        xT = np.zeros((DIM, MAXC), np.float32)
        xT[:, valid] = x[pos[valid]].T
        vee = np.zeros((MAXC, DIM), np.float32)
        vee[valid] = ve[pos[valid]] * lam1

        th = rows.astype(np.float32)[:, None] * freqs[None, :]  # [MAXC, 64]
        cosT = np.cos(th, dtype=np.float32).T.copy()
        sinT = np.sin(th, dtype=np.float32).T.copy()
        cosT[:, ~valid] = 1.0
        sinT[:, ~valid] = 0.0
        cosT = np.vstack([cosT, cosT])            # [128, MAXC]
        sinT = np.vstack([sinT, -sinT])           # rows 64-127 carry the -sin

        qpos = pos[CO:CO + MAXQ]
        qvalid = qpos >= 0
        dj = docs[rows]
        di = docs[np.where(qvalid, qpos, 0)]
        m = (valid[:, None] & qvalid[None, :]
             & (dj[:, None] == di[None, :])
             & (pos[:, None] <= qpos[None, :]))
        in_maps.append({
            "xT": _round_tf32(xT), "wq": wq, "wk": wk, "wv": wv, "cw": cw,
            "ve": vee, "cosT": cosT, "sinT": sinT,
            "mask": m.astype(bf16), "c012": c012,
            "epsb": np.full((128, 1), HD * EPS, np.float32),
        })
    return in_maps


def _run(x, ve, qkv_w, lambdas, c_proj_w, docs, trace=False):
    docs = np.asarray(docs)
    plan = _plan(docs)
    plan["docs"] = docs
    key = (plan["MAXQ"], plan["MAXC"], plan["CO"])
    if key not in _CACHE:
        _CACHE[key] = _build(*key)
    nc = _CACHE[key]
    in_maps = _prep(x, ve, qkv_w, lambdas, c_proj_w, plan)
    if trace:
        _install_ntff_shim()
    from concourse.bass_utils import run_bass_kernel_spmd
    res = run_bass_kernel_spmd(nc, in_maps, core_ids=list(range(NCORE)), trace=trace)
    out = np.zeros((T, DIM), np.float32)
    for c, (s, e) in enumerate(plan["chunks"]):
        out[s:e] = res.results[c]["outT"].T[:e - s]
    return out.reshape(1, T, DIM), res


def kernel(x, ve, qkv_w, lambdas, c_proj_w, docs):
    out, _ = _run(x, ve, qkv_w, lambdas, c_proj_w, docs, trace=False)
    return out


# revision 9
# speedup vs baseline: 1.4840x; 1.4840x over previous
"""Doc-causal self-attention (B=1, T=4096, DIM=1024, H=8, HD=128) on 8 TRN2
NeuronCores.

Strategy: sequence-parallel with doc-aligned chunks. docs is sorted, so T
splits into ~8 documents; each core takes one contiguous chunk of rows cut at
(or near) document boundaries. Doc-causal attention never crosses a document,
so each core's rows attend only to rows inside its own chunk context (chunk +
intra-doc prefix when a cut lands mid-document). Every core independently
computes qkv projection, rms-norm, rotary, masked softmax attention and its
c_proj output rows -- zero inter-core collectives. Host gathers per-core
output chunks.

Numerics: matmuls run in float32r (TF32, full TensorE rate at N>=256);
host pre-rounds matmul inputs to TF32. Softmax skips max-subtraction
(|logits| <= 0.12*128 by Cauchy-Schwarz after rms-norm). Masking is
multiplicative on exp(scores): mask value 0/1, applied post-exp.
rms scale for q is folded into q; for k it is applied per-partition via the
activation scale operand of the exp (ACT computes exp(in*scale)).
"""
import contextlib
import ctypes
import sys
import types

import numpy as np

T, DIM, H, HD = 4096, 1024, 8, 128
NCORE = 8
ATTN_SCALE = 0.12
EPS = float(np.finfo(np.float32).eps)

_CACHE = {}


def _round_tf32(a):
    u = np.ascontiguousarray(a, dtype=np.float32).view(np.uint32).astype(np.uint64)
    r = ((u + 0x1000 + ((u >> 13) & 1)) & ~np.uint64(0x1FFF)).astype(np.uint32)
    return r.view(np.float32)


# ---------------------------------------------------------------- ntff shim
def _install_ntff_shim():
    try:
        import antenv.axon_hooks  # noqa: F401
        return
    except ImportError:
        pass
    import antenv

    so_path = "/opt/axon/libaxon_pjrt.so"

    def _hook_factory():
        lib = ctypes.CDLL(so_path)
        if not hasattr(lib, "axon_start_nrt_profile"):
            return None
        lib.axon_start_nrt_profile.argtypes = [ctypes.POINTER(ctypes.c_int64), ctypes.c_size_t]
        lib.axon_start_nrt_profile.restype = ctypes.c_int64
        lib.axon_stop_nrt_profile.argtypes = [ctypes.c_char_p]
        lib.axon_stop_nrt_profile.restype = ctypes.c_int64

        @contextlib.contextmanager
        def _hook(output_dir, device_ids):
            import jax

            jax.devices()
            if device_ids:
                ids = (ctypes.c_int64 * len(device_ids))(*device_ids)
                rc = lib.axon_start_nrt_profile(ids, len(device_ids))
            else:
                rc = lib.axon_start_nrt_profile(None, 0)
            if rc != 0:
                raise RuntimeError(f"axon_start_nrt_profile rc={rc}")
            try:
                yield
            finally:
                n = lib.axon_stop_nrt_profile(str(output_dir).encode())
                print(f"profile: {n} file(s) written to {output_dir}", file=sys.stderr)

        return _hook

    mod = types.ModuleType("antenv.axon_hooks")
    _h = [_hook_factory()]
    mod.set_axon_ntff_profile_hook = lambda h: _h.__setitem__(0, h)
    mod.get_axon_ntff_profile_hook = lambda: _h[0]
    sys.modules["antenv.axon_hooks"] = mod
    antenv.axon_hooks = mod


# ---------------------------------------------------------------- planning
def _ceil128(n):
    return ((n + 127) // 128) * 128


def _plan(docs):
    docs = np.asarray(docs)
    bounds = [0] + list((np.where(np.diff(docs) != 0)[0] + 1).tolist()) + [T]
    doc_start = np.zeros(T, np.int64)
    for b0, b1 in zip(bounds[:-1], bounds[1:]):
        doc_start[b0:b1] = b0
    base = T // NCORE
    cuts = [0]
    for c in range(1, NCORE):
        tgt = c * base
        b = min(bounds, key=lambda v: abs(v - tgt))
        s = b if abs(b - tgt) <= 192 else tgt
        s = int(min(max(s, cuts[-1] + 1), T - 1))
        cuts.append(s)
    cuts.append(T)
    chunks = [(cuts[i], cuts[i + 1]) for i in range(NCORE)]
    ctx_starts = [int(doc_start[s]) for s, _ in chunks]
    MAXQ = _ceil128(max(e - s for s, e in chunks))
    PFX = _ceil128(max(s - cs for (s, _), cs in zip(chunks, ctx_starts)))
    MAXC = MAXQ + PFX
    CO = PFX
    return {"chunks": chunks, "ctx_starts": ctx_starts, "MAXQ": MAXQ,
            "MAXC": MAXC, "CO": CO, "doc_start": doc_start}


def _nchunks(width):
    n = (width + 511) // 512
    while width % n:
        n += 1
    return n, width // n


# ---------------------------------------------------------------- device
def _build(MAXQ, MAXC, CO):
    import concourse.bacc as bacc
    import concourse.mybir as mybir
    from concourse import tile, bass_isa

    dt = mybir.dt
    AF = mybir.ActivationFunctionType
    QT = MAXQ // 128
    TC = MAXC // 128
    NCH, CH = _nchunks(MAXQ)     # q-side chunks (e.g. 2 x 320)
    NCC, CC = _nchunks(MAXC)     # ctx-side chunks
    PSW = 512 * max(NCH, NCC, 2)  # psum tile free width (bank-strided chunks)

    nc = bacc.Bacc("TRN2", target_bir_lowering=False)

    xT_d = nc.dram_tensor("xT", [DIM, MAXC], dt.bfloat16, kind="ExternalInput")
    wq_d = nc.dram_tensor("wq", [DIM, DIM], dt.bfloat16, kind="ExternalInput")
    wk_d = nc.dram_tensor("wk", [DIM, DIM], dt.bfloat16, kind="ExternalInput")
    wv_d = nc.dram_tensor("wv", [DIM, DIM], dt.bfloat16, kind="ExternalInput")
    cw_d = nc.dram_tensor("cw", [DIM, DIM], dt.bfloat16, kind="ExternalInput")
    ve_d = nc.dram_tensor("ve", [MAXC, DIM], dt.float32, kind="ExternalInput")
    cos_d = nc.dram_tensor("cosT", [128, MAXC], dt.float32, kind="ExternalInput")
    sin_d = nc.dram_tensor("sinT", [128, MAXC], dt.float32, kind="ExternalInput")
    msk_d = nc.dram_tensor("mask", [MAXC, MAXQ], dt.bfloat16, kind="ExternalInput")
    c12_d = nc.dram_tensor("c012", [1, 1], dt.bfloat16, kind="ExternalInput")
    eps_d = nc.dram_tensor("epsb", [128, 1], dt.float32, kind="ExternalInput")
    onb_d = nc.dram_tensor("ones_bf", [128, 1], dt.bfloat16, kind="ExternalInput")
    on1_d = nc.dram_tensor("ones1", [1, 128], dt.float32, kind="ExternalInput")
    out_d = nc.dram_tensor("outT", [DIM, MAXQ], dt.float32, kind="ExternalOutput")

    add, mult, sub = mybir.AluOpType.add, mybir.AluOpType.mult, mybir.AluOpType.subtract

    with tile.TileContext(nc) as tc:
        with contextlib.ExitStack() as ex:
            pers = ex.enter_context(tc.tile_pool(name="pers", bufs=1))
            strm = ex.enter_context(tc.tile_pool(name="strm", bufs=4))
            nrm = ex.enter_context(tc.tile_pool(name="nrm", bufs=2))
            petp = ex.enter_context(tc.tile_pool(name="et", bufs=3))
            pps = ex.enter_context(tc.tile_pool(name="ps", bufs=2, space="PSUM"))
            psm = ex.enter_context(tc.tile_pool(name="psm", bufs=1, space="PSUM"))
            pyy = ex.enter_context(tc.tile_pool(name="py", bufs=1, space="PSUM"))

            # ---- persistent loads (x + qk weights interleaved so head 0
            # can start as soon as the first c-tile pair lands)
            xs, wqs, wks, cws = [], [], [], []
            for c in range(8):
                t = pers.tile([128, MAXC], dt.bfloat16, tag=f"x{c}", name=f"x{c}")
                nc.sync.dma_start(t[:], xT_d[c * 128:(c + 1) * 128, :])
                xs.append(t)
                wt = pers.tile([128, DIM], dt.bfloat16, tag=f"wq{c}", name=f"wqs{c}")
                nc.sync.dma_start(wt[:], wq_d[c * 128:(c + 1) * 128, :])
                wqs.append(wt)
                kt = pers.tile([128, DIM], dt.bfloat16, tag=f"wk{c}", name=f"wks{c}")
                nc.sync.dma_start(kt[:], wk_d[c * 128:(c + 1) * 128, :])
                wks.append(kt)
            for c in range(8):
                ct = pers.tile([128, DIM], dt.bfloat16, tag=f"cw{c}", name=f"cws{c}")
                nc.gpsimd.dma_start(ct[:], cw_d[c * 128:(c + 1) * 128, :])
                cws.append(ct)
            cosT = pers.tile([128, MAXC], dt.float32, tag="cos")
            sinT = pers.tile([128, MAXC], dt.float32, tag="sin")
            nc.sync.dma_start(cosT[:], cos_d[:])
            nc.sync.dma_start(sinT[:], sin_d[:])
            c12 = pers.tile([1, 1], dt.bfloat16, tag="c12")
            nc.sync.dma_start(c12[:], c12_d[:])
            epsb = pers.tile([128, 1], dt.float32, tag="epsb")
            nc.sync.dma_start(epsb[:], eps_d[:])
            onb = pers.tile([128, 1], dt.bfloat16, tag="onb")
            nc.sync.dma_start(onb[:], onb_d[:])
            on1 = pers.tile([1, 128], dt.float32, tag="on1")
            nc.sync.dma_start(on1[:], on1_d[:])
            msks = []
            for jt in range(TC):
                t = pers.tile([128, MAXQ], dt.bfloat16, tag=f"m{jt}")
                nc.gpsimd.dma_start(t[:], msk_d[jt * 128:(jt + 1) * 128, :])
                msks.append(t)

            qrs = [pers.tile([128, MAXQ], dt.bfloat16, tag=f"qr{h}", name=f"qr{h}") for h in range(H)]
            krs = [pers.tile([128, MAXC], dt.bfloat16, tag=f"kr{h}", name=f"kr{h}") for h in range(H)]
            vsb = [pers.tile([128, DIM], dt.bfloat16, tag=f"v{jt}", name=f"v{jt}") for jt in range(TC)]
            yts = [pers.tile([128, MAXQ], dt.bfloat16, tag=f"y{h}", name=f"yt{h}") for h in range(H)]
            sks = [pers.tile([128, TC], dt.float32, tag=f"sk{h}", name=f"sk{h}") for h in range(H)]
            skreps = [pers.tile([1, MAXC], dt.bfloat16, tag=f"skr{h}", name=f"skr{h}") for h in range(H)]

            # ---- qk projection + norm + rotary, per head
            for h in range(H):
                with nc.named_scope(f"proj{h}"):
                    qps = pps.tile([128, PSW], dt.float32, tag="mm")
                    kps = pps.tile([128, PSW], dt.float32, tag="mm")
                    for c in range(8):
                        for b in range(NCH):
                            nc.tensor.matmul(qps[:, b * 512:b * 512 + CH],
                                             wqs[c][:, h * 128:(h + 1) * 128],
                                             xs[c][:, CO + b * CH:CO + (b + 1) * CH],
                                             start=(c == 0), stop=(c == 7))
                    for c in range(8):
                        for b in range(NCC):
                            nc.tensor.matmul(kps[:, b * 512:b * 512 + CC],
                                             wks[c][:, h * 128:(h + 1) * 128],
                                             xs[c][:, b * CC:(b + 1) * CC],
                                             start=(c == 0), stop=(c == 7))
                with nc.named_scope(f"norm{h}"):
                    # q: rms scale folded into q before rotary
                    sq = nrm.tile([128, MAXQ], dt.bfloat16, tag="sq")
                    for b in range(NCH):
                        nc.scalar.activation(sq[:, b * CH:(b + 1) * CH],
                                             qps[:, b * 512:b * 512 + CH], AF.Square)
                    ssq = psm.tile([128, PSW], dt.float32, tag="small")
                    for b in range(NCH):
                        nc.tensor.matmul(ssq[0:1, b * 512:b * 512 + CH], onb[:],
                                         sq[:, b * CH:(b + 1) * CH], start=True, stop=True)
                    rms = nrm.tile([1, MAXC], dt.float32, tag="row")
                    for b in range(NCH):
                        nc.scalar.activation(rms[0:1, b * CH:(b + 1) * CH],
                                             ssq[0:1, b * 512:b * 512 + CH], AF.Sqrt,
                                             bias=epsb[0:1, :])
                    nc.vector.reciprocal_approx_fast(out=rms[0:1, 0:MAXQ], in_=rms[0:1, 0:MAXQ])
                    bcp = psm.tile([128, PSW], dt.float32, tag="small")
                    for b in range(NCH):
                        nc.tensor.matmul(bcp[:, b * 512:b * 512 + CH], on1[:],
                                         rms[0:1, b * CH:(b + 1) * CH], start=True, stop=True)
                    bcs = nrm.tile([128, MAXQ], dt.float32, tag="bc")
                    for b in range(NCH):
                        nc.scalar.activation(bcs[:, b * CH:(b + 1) * CH],
                                             bcp[:, b * 512:b * 512 + CH], AF.Copy)
                    qs = nrm.tile([128, MAXQ], dt.float32, tag="qs")
                    for b in range(NCH):
                        nc.vector.tensor_mul(qs[:, b * CH:(b + 1) * CH],
                                             qps[:, b * 512:b * 512 + CH],
                                             bcs[:, b * CH:(b + 1) * CH])
                    qsw = nrm.tile([128, MAXQ], dt.float32, tag="qsw")
                    nc.gpsimd.dma_start(qsw[0:64, :], qs[64:128, :])
                    nc.gpsimd.dma_start(qsw[64:128, :], qs[0:64, :])
                    ta = nrm.tile([128, MAXC], dt.float32, tag="ta")
                    tb = nrm.tile([128, MAXC], dt.float32, tag="tb")
                    nc.vector.tensor_mul(ta[:, 0:MAXQ], qs[:], cosT[:, CO:CO + MAXQ])
                    nc.vector.tensor_mul(tb[:, 0:MAXQ], qsw[:], sinT[:, CO:CO + MAXQ])
                    nc.vector.tensor_add(qrs[h][:], ta[:, 0:MAXQ], tb[:, 0:MAXQ])
                    # k: evacuate, square, rms kept separate (applied at exp)
                    ks = nrm.tile([128, MAXC], dt.float32, tag="ks", bufs=1)
                    for b in range(NCC):
                        nc.scalar.activation(ks[:, b * CC:(b + 1) * CC],
                                             kps[:, b * 512:b * 512 + CC], AF.Copy)
                    sqk = nrm.tile([128, MAXC], dt.bfloat16, tag="sq")
                    nc.scalar.activation(sqk[:], ks[:], AF.Square)
                    ssqk = psm.tile([128, PSW], dt.float32, tag="small")
                    for b in range(NCC):
                        nc.tensor.matmul(ssqk[0:1, b * 512:b * 512 + CC], onb[:],
                                         sqk[:, b * CC:(b + 1) * CC], start=True, stop=True)
                    skf = nrm.tile([1, MAXC], dt.float32, tag="row")
                    for b in range(NCC):
                        nc.scalar.activation(skf[0:1, b * CC:(b + 1) * CC],
                                             ssqk[0:1, b * 512:b * 512 + CC], AF.Sqrt,
                                             bias=epsb[0:1, :])
                    nc.vector.reciprocal_approx_fast(out=skf[0:1, :], in_=skf[0:1, :])
                    nc.scalar.activation(skreps[h][:], skf[0:1, :], AF.Copy)
                    ksw = nrm.tile([128, MAXC], dt.float32, tag="ksw")
                    nc.gpsimd.dma_start(ksw[0:64, :], ks[64:128, :])
                    nc.gpsimd.dma_start(ksw[64:128, :], ks[0:64, :])
                    ka = nrm.tile([128, MAXC], dt.float32, tag="ta")
                    kb = nrm.tile([128, MAXC], dt.float32, tag="tb")
                    nc.vector.tensor_mul(ka[:], ks[:], cosT[:])
                    nc.vector.tensor_mul(kb[:], ksw[:], sinT[:])
                    nc.vector.tensor_add(krs[h][:], ka[:], kb[:])

            # ---- v projection + ve lerp (2 jt-groups, streamed wv)
            with nc.named_scope("vproj"):
                groups = [list(range(TC))[i::2] for i in range(2)]
                groups = [list(range(TC))[i:i + 2] for i in range(0, TC, 2)]
                for grp in groups:
                    vpss = [pps.tile([128, PSW], dt.float32, tag="mm", name=f"vps{jt}") for jt in grp]
                    for c in range(8):
                        wvt = strm.tile([128, DIM], dt.bfloat16, tag="wv", bufs=2)
                        nc.gpsimd.dma_start(wvt[:], wv_d[c * 128:(c + 1) * 128, :])
                        for vps, jt in zip(vpss, grp):
                            for b2 in range(2):
                                nc.tensor.matmul(vps[:, b2 * 512:(b2 + 1) * 512],
                                                 xs[c][:, jt * 128:(jt + 1) * 128],
                                                 wvt[:, b2 * 512:(b2 + 1) * 512],
                                                 start=(c == 0), stop=(c == 7))
                    for vps, jt in zip(vpss, grp):
                        vet = strm.tile([128, DIM], dt.float32, tag="ve", bufs=2)
                        nc.sync.dma_start(vet[:], ve_d[jt * 128:(jt + 1) * 128, :])
                        nc.vector.tensor_add(vsb[jt][:], vps[:, 0:1024], vet[:])

            # ---- s_k column transpose (PE), per head
            with nc.named_scope("sk"):
                for h in range(H):
                    skp = pps.tile([128, PSW], dt.float32, tag="mm")
                    for jt in range(TC):
                        nc.tensor.matmul(skp[:, jt:jt + 1], skreps[h][0:1, jt * 128:(jt + 1) * 128],
                                         c12[:], start=True, stop=True)
                    nc.vector.tensor_copy(out=sks[h][:], in_=skp[:, 0:TC])

            # ---- attention, per head (QK / mask-mul / exp / PV pipelined)
            for h in range(H):
                with nc.named_scope(f"attn{h}"):
                    yps = pyy.tile([128, PSW], dt.float32, tag="y")
                    rsp = psm.tile([128, PSW], dt.float32, tag="small")
                    for jt in range(TC):
                        stp = pps.tile([128, PSW], dt.float32, tag="mm")
                        for b in range(NCH):
                            nc.tensor.matmul(stp[:, b * 512:b * 512 + CH],
                                             krs[h][:, jt * 128:(jt + 1) * 128],
                                             qrs[h][:, b * CH:(b + 1) * CH],
                                             start=True, stop=True)
                        et = petp.tile([128, MAXQ], dt.bfloat16, tag="et")
                        for b in range(NCH):
                            nc.scalar.activation(et[:, b * CH:(b + 1) * CH],
                                                 stp[:, b * 512:b * 512 + CH], AF.Exp,
                                                 scale=sks[h][:, jt:jt + 1])
                        nc.vector.tensor_mul(et[:], et[:], msks[jt][:])
                        for b in range(NCH):
                            nc.tensor.matmul(rsp[0:1, b * 512:b * 512 + CH], onb[:],
                                             et[:, b * CH:(b + 1) * CH],
                                             start=(jt == 0), stop=(jt == TC - 1))
                        for b in range(NCH):
                            nc.tensor.matmul(yps[:, b * 512:b * 512 + CH],
                                             vsb[jt][:, h * 128:(h + 1) * 128],
                                             et[:, b * CH:(b + 1) * CH],
                                             start=(jt == 0), stop=(jt == TC - 1))
                    rrow = nrm.tile([1, MAXC], dt.float32, tag="row")
                    for b in range(NCH):
                        nc.scalar.activation(rrow[0:1, b * CH:(b + 1) * CH],
                                             rsp[0:1, b * 512:b * 512 + CH], AF.Copy)
                    nc.vector.reciprocal_approx_fast(out=rrow[0:1, 0:MAXQ], in_=rrow[0:1, 0:MAXQ])
                    rbp = psm.tile([128, PSW], dt.float32, tag="small")
                    for b in range(NCH):
                        nc.tensor.matmul(rbp[:, b * 512:b * 512 + CH], on1[:],
                                         rrow[0:1, b * CH:(b + 1) * CH], start=True, stop=True)
                    rbs = nrm.tile([128, MAXQ], dt.float32, tag="bc")
                    for b in range(NCH):
                        nc.scalar.activation(rbs[:, b * CH:(b + 1) * CH],
                                             rbp[:, b * 512:b * 512 + CH], AF.Copy)
                    for b in range(NCH):
                        nc.vector.tensor_mul(yts[h][:, b * CH:(b + 1) * CH],
                                             yps[:, b * 512:b * 512 + CH],
                                             rbs[:, b * CH:(b + 1) * CH])

            # ---- c_proj
            with nc.named_scope("cproj"):
                for e in range(8):
                    ops = pps.tile([128, PSW], dt.float32, tag="mm")
                    for h in range(H):
                        for b in range(NCH):
                            nc.tensor.matmul(ops[:, b * 512:b * 512 + CH],
                                             cws[h][:, e * 128:(e + 1) * 128],
                                             yts[h][:, b * CH:(b + 1) * CH],
                                             start=(h == 0), stop=(h == 7))
                    osb = nrm.tile([128, MAXQ], dt.float32, tag="osb")
                    for b in range(NCH):
                        nc.scalar.activation(osb[:, b * CH:(b + 1) * CH],
                                             ops[:, b * 512:b * 512 + CH], AF.Copy)
                    nc.sync.dma_start(out_d[e * 128:(e + 1) * 128, :], osb[:])

    nc.compile()
    return nc


# ---------------------------------------------------------------- host glue
def _prep(x, ve, qkv_w, lambdas, c_proj_w, plan):
    x = np.asarray(x, np.float32).reshape(T, DIM)
    ve = np.asarray(ve, np.float32).reshape(T, DIM)
    qkv_w = np.asarray(qkv_w, np.float32)
    lam0, lam1 = [float(v) for v in np.asarray(lambdas, np.float32)]
    MAXQ, MAXC, CO = plan["MAXQ"], plan["MAXC"], plan["CO"]
    docs = plan["docs"]

    import ml_dtypes
    bf16 = ml_dtypes.bfloat16
    wq = np.ascontiguousarray(qkv_w[0].T).astype(bf16)
    wk = np.ascontiguousarray(qkv_w[1].T).astype(bf16)
    wv = np.ascontiguousarray(qkv_w[2].T * lam0).astype(bf16)
    cw = np.ascontiguousarray(np.asarray(c_proj_w, np.float32).T).astype(bf16)
    c012 = np.full((1, 1), ATTN_SCALE * HD, np.float32).astype(bf16)

    # rotary tables (global positions; freqs as in reference)
    lin = np.linspace(0.0, 1.0, HD // 4).astype(np.float32)
    freqs = np.power(np.float32(1.0 / 1024.0), lin).astype(np.float32)
    freqs = np.concatenate([freqs, np.zeros(HD // 4, np.float32)])

    in_maps = []
    for (s, e), cs in zip(plan["chunks"], plan["ctx_starts"]):
        clen, plen = e - s, s - cs
        pos = np.full(MAXC, -1, np.int64)
        pos[CO - plen:CO + clen] = np.arange(cs, e)
        valid = pos >= 0
        rows = np.where(valid, pos, 0)

        xT = np.zeros((DIM, MAXC), np.float32)
        xT[:, valid] = x[pos[valid]].T
        vee = np.zeros((MAXC, DIM), np.float32)
        vee[valid] = ve[pos[valid]] * lam1

        th = rows.astype(np.float32)[:, None] * freqs[None, :]  # [MAXC, 64]
        cosT = np.cos(th, dtype=np.float32).T.copy()
        sinT = np.sin(th, dtype=np.float32).T.copy()
        cosT[:, ~valid] = 1.0
        sinT[:, ~valid] = 0.0
        cosT = np.vstack([cosT, cosT])            # [128, MAXC]
        sinT = np.vstack([sinT, -sinT])           # rows 64-127 carry the -sin

        qpos = pos[CO:CO + MAXQ]
        qvalid = qpos >= 0
        dj = docs[rows]
        di = docs[np.where(qvalid, qpos, 0)]
        m = (valid[:, None] & qvalid[None, :]
             & (dj[:, None] == di[None, :])
             & (pos[:, None] <= qpos[None, :]))
        in_maps.append({
            "xT": xT.astype(bf16), "wq": wq, "wk": wk, "wv": wv, "cw": cw,
            "ve": vee, "cosT": cosT, "sinT": sinT,
            "mask": m.astype(bf16), "c012": c012,
            "epsb": np.full((128, 1), HD * EPS, np.float32),
            "ones_bf": np.ones((128, 1), bf16),
            "ones1": np.ones((1, 128), np.float32),
        })
    return in_maps


def _run(x, ve, qkv_w, lambdas, c_proj_w, docs, trace=False):
    docs = np.asarray(docs)
    plan = _plan(docs)
    plan["docs"] = docs
    key = (plan["MAXQ"], plan["MAXC"], plan["CO"])
    if key not in _CACHE:
        _CACHE[key] = _build(*key)
    nc = _CACHE[key]
    in_maps = _prep(x, ve, qkv_w, lambdas, c_proj_w, plan)
    if trace:
        _install_ntff_shim()
    from concourse.bass_utils import run_bass_kernel_spmd
    res = run_bass_kernel_spmd(nc, in_maps, core_ids=list(range(NCORE)), trace=trace)
    out = np.zeros((T, DIM), np.float32)
    for c, (s, e) in enumerate(plan["chunks"]):
        out[s:e] = res.results[c]["outT"].T[:e - s]
    return out.reshape(1, T, DIM), res


def kernel(x, ve, qkv_w, lambdas, c_proj_w, docs):
    out, _ = _run(x, ve, qkv_w, lambdas, c_proj_w, docs, trace=False)
    return out
